# revision 14
# baseline (speedup 1.0000x reference)
"""Trainium2 Bass kernel for DyDepthwiseConvAtten.

Computation (per (b, n) row r of C=256 channels):
  w[r, k]  = sum_c q[r, c] * W_w[k, c] + b_w[k]          (k = 0..2)
  x[r, c]  = sum_k w[r, k] * vpad[r, c + k]               (3-tap depthwise conv, 'same')
  out[r,c] = (x - mean_c(x)) * rsqrt(var_c(x) + eps) * gamma[c] + beta[c]

Strategy: pure data-parallel over batch across 8 cores. Rows (b*n flattened)
live on SBUF partitions; compute in tiles of 128 rows x 256 channels,
DMA-batched G row-tiles at a time (HWDGE descriptor-generation is ~625ns
per DMA instruction, so few big DMAs beat many small ones).

  - w via TensorE: host pre-transposes q so each tile's qT chunk [128c, 128r]
    is the (self-loading) stationary operand against W_w^T [128c, 3],
    accumulating the two c-chunks in PSUM.
  - conv via TensorE: x_psum += diag(w_k) @ v_shifted_k for k = 0..2, where
    diag(w_k) = identity * w_k is built with one per-partition tensor_scalar
    each.  In "f32r"/"bf16" modes the conv matmuls stream 1 column/cycle
    (4x the fp32 rate).
  - LayerNorm: bn_stats/bn_aggr on VectorE, sqrt on ScalarE, reciprocal on
    VectorE, normalize as one ScalarE activation y = Id(x * rs + (-mu * rs)).

Modes (BASS_DYCONV_MODE):
  f32   — everything fp32 (exact, conv matmuls at 4 cycles/row)
  f32r  — conv matmuls in float32r (fp32 bits, reduced-precision multiply)
  bf16  — value + conv in bf16 (halves v DMA traffic)
  bf16q — bf16 value/conv AND bf16 query/W_w (halves q DMA traffic too)
"""

import os
from contextlib import ExitStack

import numpy as np
import ml_dtypes

import concourse.bacc as bacc
import concourse.bass as bass
import concourse.tile as tile
from concourse import mybir
from concourse.bass_utils import run_bass_kernel_spmd
from concourse.masks import make_identity

B, N, C, K = 1024, 100, 256, 3
N_CORES = 8
B_PER_CORE = B // N_CORES        # 128
ROWS = B_PER_CORE * N            # 12800 rows per core
P = 128                          # partitions (rows per tile)
N_ROW_TILES = ROWS // P          # 100
G = int(os.environ.get("BASS_DYCONV_G", "10"))  # row-tiles per DMA super-tile
assert N_ROW_TILES % G == 0 and G % 2 == 0
LN_EPS = 1e-5
F32 = mybir.dt.float32
F32R = mybir.dt.float32r
BF16 = mybir.dt.bfloat16

CONV_MODE = os.environ.get("BASS_DYCONV_MODE", "f32")
TRACE = bool(int(os.environ.get("BASS_DYCONV_TRACE", "0")))

LAST_EXEC_NS = None
LAST_RESULTS = None

_cache = {}


def _dtypes(mode):
    """(q/wwt dtype, v dtype, dk dtype) per conv mode."""
    if mode == "f32":
        return F32, F32, F32
    if mode == "f32r":
        return F32, F32R, F32R
    if mode == "bf16":
        return F32, BF16, BF16
    if mode == "bf16q":
        return BF16, BF16, BF16
    raise ValueError(mode)


def _build(conv_mode: str, apply_affine: bool, add_bias: bool,
           loop_n: int = 1):
    nc = bacc.Bacc("TRN2", target_bir_lowering=False, debug=False)
    qdt, vdt, _ = _dtypes(conv_mode)

    # qT: per-core transposed query, chunked: [2, 128c, ROWS]
    qT = nc.dram_tensor("qt", [2, P, ROWS], qdt, kind="ExternalInput")
    v = nc.dram_tensor("v", [ROWS, C], vdt, kind="ExternalInput")
    # W_w^T chunks: [2, 128c, K]
    wwt = nc.dram_tensor("wwt", [2, P, K], qdt, kind="ExternalInput")
    out = nc.dram_tensor("out", [ROWS, C], F32, kind="ExternalOutput")
    gamma = beta = bwb = None
    if apply_affine:
        gamma = nc.dram_tensor("gamma", [1, C], F32, kind="ExternalInput")
        beta = nc.dram_tensor("beta", [1, C], F32, kind="ExternalInput")
    if add_bias:
        bwb = nc.dram_tensor("bw", [1, K], F32, kind="ExternalInput")

    def emit():
        _emit(ctx, tc, qT.ap(), v.ap(), wwt.ap(), out.ap(),
              gamma.ap() if gamma is not None else None,
              beta.ap() if beta is not None else None,
              bwb.ap() if bwb is not None else None,
              conv_mode)

    with tile.TileContext(nc) as tc, ExitStack() as ctx:
        if loop_n > 1:
            with tc.For_i(0, loop_n, 1):
                emit()
        else:
            emit()
    nc.compile()
    return nc


def _bcast_rows(ap: bass.AP, nrows: int) -> bass.AP:
    """DMA access pattern replicating a [1, F] DRAM tensor across partitions."""
    return bass.AP(tensor=ap.tensor, offset=ap.offset,
                   ap=[[0, nrows]] + list(ap.ap[1:]))


def _emit(ctx, tc, qT, v, wwt, out, gamma, beta, bwb, conv_mode):
    nc = tc.nc
    mult = mybir.AluOpType.mult
    AF = mybir.ActivationFunctionType
    qdt, vdt, kdt = _dtypes(conv_mode)

    singles = ctx.enter_context(tc.tile_pool(name="singles", bufs=1))
    pool = ctx.enter_context(tc.tile_pool(name="work", bufs=3))
    dkp = ctx.enter_context(tc.tile_pool(name="dkp", bufs=4))
    small = ctx.enter_context(tc.tile_pool(name="small", bufs=8))
    psum_x = ctx.enter_context(
        tc.tile_pool(name="psum_x", bufs=4, space=bass.MemorySpace.PSUM))
    psum_w = ctx.enter_context(
        tc.tile_pool(name="psum_w", bufs=3, space=bass.MemorySpace.PSUM))

    # one-time constants
    ident = singles.tile([P, P], F32)
    make_identity(nc, ident[:])
    wwt_sb = singles.tile([P, 2, K], qdt)
    nc.sync.dma_start(out=wwt_sb[:], in_=wwt.rearrange("a p k -> p a k"))
    eps_sb = singles.tile([P, 1], F32)
    nc.vector.memset(eps_sb[:], LN_EPS)
    if gamma is not None:
        gamma_sb = singles.tile([P, C], F32)
        nc.sync.dma_start(out=gamma_sb[:], in_=_bcast_rows(gamma, P))
        beta_sb = singles.tile([P, C], F32)
        nc.sync.dma_start(out=beta_sb[:], in_=_bcast_rows(beta, P))
    if bwb is not None:
        bw_sb = singles.tile([P, K], F32)
        nc.sync.dma_start(out=bw_sb[:], in_=_bcast_rows(bwb, P))

    for st in range(N_ROW_TILES // G):
        r0 = st * G * P

        # ---- batched loads ----
        qt_t = pool.tile([P, 2, G * P], qdt, tag="qt")
        nc.sync.dma_start(out=qt_t[:],
                          in_=qT[:, :, r0:r0 + G * P].rearrange(
                              "a c r -> c a r"))
        v_t = pool.tile([P, G, C], vdt, tag="vt")
        nc.sync.dma_start(out=v_t[:],
                          in_=v[r0:r0 + G * P, :].rearrange(
                              "(g p) c -> p g c", p=P))
        y_t = pool.tile([P, G, C], F32, tag="y")

        # sub-tiles processed in pairs so the small LN ops (sqrt,
        # reciprocal) batch across two row-tiles; PSUM tiles stay
        # per-sub-tile (separate banks — same-bank PE-write/engine-read
        # is a hardware hazard)
        for h in range(G // 2):
            xs = []
            mv = small.tile([P, 2, 2], F32, tag="mv")
            for j in range(2):
                g = 2 * h + j
                qt_g = qt_t[:, :, g * P:(g + 1) * P]
                v_g = v_t[:, g, :]

                # dynamic weights w[r, k] on TensorE
                w_ps = psum_w.tile([P, K], F32, tag="w_ps")
                nc.tensor.matmul(w_ps[:], lhsT=qt_g[:, 0, :],
                                 rhs=wwt_sb[:, 0, :], start=True, stop=False)
                nc.tensor.matmul(w_ps[:], lhsT=qt_g[:, 1, :],
                                 rhs=wwt_sb[:, 1, :], start=False, stop=True)
                w_sb = small.tile([P, K], F32, tag="w_sb")
                if bwb is not None:
                    nc.vector.tensor_add(w_sb[:], w_ps[:], bw_sb[:])
                else:
                    nc.scalar.copy(w_sb[:], w_ps[:])

                # diagonal stationaries diag(w_k) = identity * w_k
                # (k=0,2 on GpSimd, k=1 on VectorE — spreads the load)
                dk = dkp.tile([P, K, P], kdt, tag="dk")
                nc.gpsimd.tensor_scalar_mul(dk[:, 0, :], ident[:],
                                            w_sb[:, 0:1])
                nc.vector.tensor_scalar_mul(dk[:, 1, :], ident[:],
                                            w_sb[:, 1:2])
                nc.gpsimd.tensor_scalar_mul(dk[:, 2, :], ident[:],
                                            w_sb[:, 2:3])
                # 'same' padding without a padded buffer: the aligned k=1
                # tap covers all C columns (start=True clears PSUM); k=0 /
                # k=2 accumulate over their valid C-1 column subranges.
                x_ps = psum_x.tile([P, C], F32, tag="x_ps")
                nc.tensor.matmul(x_ps[:], lhsT=dk[:, 1, :], rhs=v_g[:],
                                 start=True, stop=False)
                nc.tensor.matmul(x_ps[:, 1:C], lhsT=dk[:, 0, :],
                                 rhs=v_g[:, 0:C - 1], start=False,
                                 stop=False)
                nc.tensor.matmul(x_ps[:, 0:C - 1], lhsT=dk[:, 2, :],
                                 rhs=v_g[:, 1:C], start=False, stop=True)
                xs.append(x_ps)

                # LayerNorm stats
                stats = small.tile([P, 6], F32, tag="stats")
                nc.vector.bn_stats(out=stats[:], in_=x_ps[:])
                nc.vector.bn_aggr(out=mv[:, j, :], in_=stats[:])

            # batched small LN ops across the pair
            rs = small.tile([P, 2], F32, tag="rs")
            nc.scalar.activation(rs[:], mv[:, :, 1], AF.Sqrt, bias=eps_sb[:])
            nc.vector.reciprocal(rs[:], rs[:])
            nbias = small.tile([P, 2], F32, tag="nbias")
            for j in range(2):
                nc.gpsimd.tensor_scalar(out=nbias[:, j:j + 1],
                                        in0=mv[:, j, 0:1],
                                        scalar1=rs[:, j:j + 1],
                                        scalar2=-1.0, op0=mult, op1=mult)
            for j in range(2):
                g = 2 * h + j
                y_g = y_t[:, g, :]
                nc.scalar.activation(y_g, xs[j][:], AF.Identity,
                                     bias=nbias[:, j:j + 1],
                                     scale=rs[:, j:j + 1])
                if gamma is not None:
                    nc.vector.tensor_mul(y_g, y_g, gamma_sb[:])
                    nc.vector.tensor_add(y_g, y_g, beta_sb[:])

        nc.scalar.dma_start(out=out[r0:r0 + G * P, :].rearrange(
            "(g p) c -> p g c", p=P), in_=y_t[:])


def kernel(query, value, W_w, b_w, gamma, beta):
    global LAST_EXEC_NS, LAST_RESULTS

    query = np.ascontiguousarray(np.asarray(query, dtype=np.float32))
    value = np.ascontiguousarray(np.asarray(value, dtype=np.float32))
    W_w = np.ascontiguousarray(np.asarray(W_w, dtype=np.float32))
    b_w = np.asarray(b_w, dtype=np.float32)
    gamma = np.asarray(gamma, dtype=np.float32)
    beta = np.asarray(beta, dtype=np.float32)

    apply_affine = not (np.all(gamma == 1.0) and np.all(beta == 0.0))
    add_bias = bool(np.any(b_w != 0.0))

    key = (CONV_MODE, apply_affine, add_bias)
    if key not in _cache:
        _cache[key] = _build(*key)
    nc = _cache[key]

    in_maps = host_in_maps(query, value, W_w, CONV_MODE)
    if apply_affine:
        for m in in_maps:
            m["gamma"] = gamma.reshape(1, C)
            m["beta"] = beta.reshape(1, C)
    if add_bias:
        for m in in_maps:
            m["bw"] = b_w.reshape(1, K)

    res = run_bass_kernel_spmd(nc, in_maps, core_ids=list(range(N_CORES)),
                               trace=TRACE)
    LAST_EXEC_NS = res.exec_time_ns
    LAST_RESULTS = res
    out = np.empty((B, N, C), dtype=np.float32)
    for c in range(N_CORES):
        out[c * B_PER_CORE:(c + 1) * B_PER_CORE] = (
            res.results[c]["out"].reshape(B_PER_CORE, N, C))
    return out


def host_in_maps(query, value, W_w, mode):
    """Shard + lay out inputs for the 8 cores (host-side, layout only)."""
    qdt, vdt, _ = _dtypes(mode)
    qnp = ml_dtypes.bfloat16 if qdt == BF16 else np.float32
    vnp = ml_dtypes.bfloat16 if vdt == BF16 else np.float32

    wwt = np.ascontiguousarray(W_w.T.reshape(2, P, K)).astype(qnp)
    q_sh = query.reshape(N_CORES, ROWS, C)
    v_sh = value.reshape(N_CORES, ROWS, C)
    in_maps = []
    for c in range(N_CORES):
        in_maps.append({
            "qt": np.ascontiguousarray(q_sh[c].T).reshape(
                2, P, ROWS).astype(qnp),
            "v": np.ascontiguousarray(v_sh[c].astype(vnp)),
            "wwt": wwt,
        })
    return in_maps


# revision 16
# speedup vs baseline: 2.4699x; 2.4699x over previous
"""Trainium2 Bass kernel for DyDepthwiseConvAtten.

Computation (per (b, n) row r of C=256 channels):
  w[r, k]  = sum_c q[r, c] * W_w[k, c] + b_w[k]          (k = 0..2)
  x[r, c]  = sum_k w[r, k] * vpad[r, c + k]               (3-tap depthwise conv, 'same')
  out[r,c] = (x - mean_c(x)) * rsqrt(var_c(x) + eps) * gamma[c] + beta[c]

Strategy: pure data-parallel over batch across 8 cores. Rows (b*n flattened)
live on SBUF partitions; compute in tiles of 128 rows x 256 channels,
DMA-batched G row-tiles at a time (HWDGE descriptor-generation is ~625ns
per DMA instruction, so few big DMAs beat many small ones).

  - w via TensorE: host pre-transposes q so each tile's qT chunk [128c, 128r]
    is the (self-loading) stationary operand against W_w^T [128c, 3],
    accumulating the two c-chunks in PSUM.
  - conv via TensorE: x_psum += diag(w_k) @ v_shifted_k for k = 0..2, where
    diag(w_k) = identity * w_k is built with one per-partition tensor_scalar
    each.  In "f32r"/"bf16" modes the conv matmuls stream 1 column/cycle
    (4x the fp32 rate).
  - LayerNorm: bn_stats/bn_aggr on VectorE, sqrt on ScalarE, reciprocal on
    VectorE, normalize as one ScalarE activation y = Id(x * rs + (-mu * rs)).

Modes (BASS_DYCONV_MODE):
  f32   — everything fp32 (exact, conv matmuls at 4 cycles/row)
  f32r  — conv matmuls in float32r (fp32 bits, reduced-precision multiply)
  bf16  — value + conv in bf16 (halves v DMA traffic)
  bf16q — bf16 value/conv AND bf16 query/W_w (halves q DMA traffic too)
"""

import os
from contextlib import ExitStack

import numpy as np
import ml_dtypes

import concourse.bacc as bacc
import concourse.bass as bass
import concourse.tile as tile
from concourse import mybir
from concourse.bass_utils import run_bass_kernel_spmd
from concourse.masks import make_identity

B, N, C, K = 1024, 100, 256, 3
N_CORES = 8
B_PER_CORE = B // N_CORES        # 128
ROWS = B_PER_CORE * N            # 12800 rows per core
P = 128                          # partitions (rows per tile)
N_ROW_TILES = ROWS // P          # 100
G = int(os.environ.get("BASS_DYCONV_G", "10"))  # row-tiles per DMA super-tile
assert N_ROW_TILES % G == 0 and G % 2 == 0
LN_EPS = 1e-5
F32 = mybir.dt.float32
F32R = mybir.dt.float32r
BF16 = mybir.dt.bfloat16

CONV_MODE = os.environ.get("BASS_DYCONV_MODE", "f32")
TRACE = bool(int(os.environ.get("BASS_DYCONV_TRACE", "0")))

LAST_EXEC_NS = None
LAST_RESULTS = None

_cache = {}


def _dtypes(mode):
    """(q/wwt dtype, v dtype, dk dtype) per conv mode."""
    if mode == "f32":
        return F32, F32, F32
    if mode == "f32r":
        return F32, F32R, F32R
    if mode == "bf16":
        return F32, BF16, BF16
    if mode == "bf16q":
        return BF16, BF16, BF16
    raise ValueError(mode)


def _build(conv_mode: str, apply_affine: bool, add_bias: bool,
           loop_n: int = 1):
    nc = bacc.Bacc("TRN2", target_bir_lowering=False, debug=False)
    qdt, vdt, _ = _dtypes(conv_mode)

    # qT: per-core transposed query, chunked: [2, 128c, ROWS]
    qT = nc.dram_tensor("qt", [2, P, ROWS], qdt, kind="ExternalInput")
    v = nc.dram_tensor("v", [ROWS, C], vdt, kind="ExternalInput")
    # W_w^T chunks: [2, 128c, K]
    wwt = nc.dram_tensor("wwt", [2, P, K], qdt, kind="ExternalInput")
    out = nc.dram_tensor("out", [ROWS, C], F32, kind="ExternalOutput")
    gamma = beta = bwb = None
    if apply_affine:
        gamma = nc.dram_tensor("gamma", [1, C], F32, kind="ExternalInput")
        beta = nc.dram_tensor("beta", [1, C], F32, kind="ExternalInput")
    if add_bias:
        bwb = nc.dram_tensor("bw", [1, K], F32, kind="ExternalInput")

    def emit():
        _emit(ctx, tc, qT.ap(), v.ap(), wwt.ap(), out.ap(),
              gamma.ap() if gamma is not None else None,
              beta.ap() if beta is not None else None,
              bwb.ap() if bwb is not None else None,
              conv_mode)

    with tile.TileContext(nc) as tc, ExitStack() as ctx:
        if loop_n > 1:
            with tc.For_i(0, loop_n, 1):
                emit()
        else:
            emit()
    nc.compile()
    return nc


def _bcast_rows(ap: bass.AP, nrows: int) -> bass.AP:
    """DMA access pattern replicating a [1, F] DRAM tensor across partitions."""
    return bass.AP(tensor=ap.tensor, offset=ap.offset,
                   ap=[[0, nrows]] + list(ap.ap[1:]))


def _emit(ctx, tc, qT, v, wwt, out, gamma, beta, bwb, conv_mode):
    nc = tc.nc
    mult = mybir.AluOpType.mult
    AF = mybir.ActivationFunctionType
    qdt, vdt, kdt = _dtypes(conv_mode)

    singles = ctx.enter_context(tc.tile_pool(name="singles", bufs=1))
    pool = ctx.enter_context(tc.tile_pool(name="work", bufs=3))
    dkp = ctx.enter_context(tc.tile_pool(name="dkp", bufs=4))
    small = ctx.enter_context(tc.tile_pool(name="small", bufs=8))
    psum_x = ctx.enter_context(
        tc.tile_pool(name="psum_x", bufs=4, space=bass.MemorySpace.PSUM))
    psum_w = ctx.enter_context(
        tc.tile_pool(name="psum_w", bufs=3, space=bass.MemorySpace.PSUM))

    # one-time constants
    ident = singles.tile([P, P], F32)
    make_identity(nc, ident[:])
    wwt_sb = singles.tile([P, 2, K], qdt)
    nc.sync.dma_start(out=wwt_sb[:], in_=wwt.rearrange("a p k -> p a k"))
    eps_sb = singles.tile([P, 1], F32)
    nc.vector.memset(eps_sb[:], LN_EPS)
    if gamma is not None:
        gamma_sb = singles.tile([P, C], F32)
        nc.sync.dma_start(out=gamma_sb[:], in_=_bcast_rows(gamma, P))
        beta_sb = singles.tile([P, C], F32)
        nc.sync.dma_start(out=beta_sb[:], in_=_bcast_rows(beta, P))
    if bwb is not None:
        bw_sb = singles.tile([P, K], F32)
        nc.sync.dma_start(out=bw_sb[:], in_=_bcast_rows(bwb, P))

    for st in range(N_ROW_TILES // G):
        r0 = st * G * P

        # ---- batched loads ----
        qt_t = pool.tile([P, 2, G * P], qdt, tag="qt")
        nc.sync.dma_start(out=qt_t[:],
                          in_=qT[:, :, r0:r0 + G * P].rearrange(
                              "a c r -> c a r"))
        v_t = pool.tile([P, G, C], vdt, tag="vt")
        nc.sync.dma_start(out=v_t[:],
                          in_=v[r0:r0 + G * P, :].rearrange(
                              "(g p) c -> p g c", p=P))
        y_t = pool.tile([P, G, C], F32, tag="y")

        # sub-tiles processed in pairs so the small LN ops (sqrt,
        # reciprocal) batch across two row-tiles; PSUM tiles stay
        # per-sub-tile (separate banks — same-bank PE-write/engine-read
        # is a hardware hazard)
        for h in range(G // 2):
            xs = []
            mv = small.tile([P, 2, 2], F32, tag="mv")
            for j in range(2):
                g = 2 * h + j
                qt_g = qt_t[:, :, g * P:(g + 1) * P]
                v_g = v_t[:, g, :]

                # dynamic weights w[r, k] on TensorE
                w_ps = psum_w.tile([P, K], F32, tag="w_ps")
                nc.tensor.matmul(w_ps[:], lhsT=qt_g[:, 0, :],
                                 rhs=wwt_sb[:, 0, :], start=True, stop=False)
                nc.tensor.matmul(w_ps[:], lhsT=qt_g[:, 1, :],
                                 rhs=wwt_sb[:, 1, :], start=False, stop=True)
                w_sb = small.tile([P, K], F32, tag="w_sb")
                if bwb is not None:
                    nc.vector.tensor_add(w_sb[:], w_ps[:], bw_sb[:])
                else:
                    nc.scalar.copy(w_sb[:], w_ps[:])

                # diagonal stationaries diag(w_k) = identity * w_k
                # (k=0,2 on GpSimd, k=1 on VectorE — spreads the load)
                dk = dkp.tile([P, K, P], kdt, tag="dk")
                for k in range(K):
                    nc.vector.tensor_scalar_mul(dk[:, k, :], ident[:],
                                                w_sb[:, k:k + 1])
                # 'same' padding without a padded buffer: the aligned k=1
                # tap covers all C columns (start=True clears PSUM); k=0 /
                # k=2 accumulate over their valid C-1 column subranges.
                x_ps = psum_x.tile([P, C], F32, tag="x_ps")
                nc.tensor.matmul(x_ps[:], lhsT=dk[:, 1, :], rhs=v_g[:],
                                 start=True, stop=False)
                nc.tensor.matmul(x_ps[:, 1:C], lhsT=dk[:, 0, :],
                                 rhs=v_g[:, 0:C - 1], start=False,
                                 stop=False)
                nc.tensor.matmul(x_ps[:, 0:C - 1], lhsT=dk[:, 2, :],
                                 rhs=v_g[:, 1:C], start=False, stop=True)
                xs.append(x_ps)

                # LayerNorm stats
                stats = small.tile([P, 6], F32, tag="stats")
                nc.vector.bn_stats(out=stats[:], in_=x_ps[:])
                nc.vector.bn_aggr(out=mv[:, j, :], in_=stats[:])

            # batched small LN ops across the pair
            rs = small.tile([P, 2], F32, tag="rs")
            nc.scalar.activation(rs[:], mv[:, :, 1], AF.Sqrt, bias=eps_sb[:])
            nc.vector.reciprocal(rs[:], rs[:])
            nbias = small.tile([P, 2], F32, tag="nbias")
            for j in range(2):
                nc.vector.tensor_scalar(out=nbias[:, j:j + 1],
                                        in0=mv[:, j, 0:1],
                                        scalar1=rs[:, j:j + 1],
                                        scalar2=-1.0, op0=mult, op1=mult)
            for j in range(2):
                g = 2 * h + j
                y_g = y_t[:, g, :]
                nc.scalar.activation(y_g, xs[j][:], AF.Identity,
                                     bias=nbias[:, j:j + 1],
                                     scale=rs[:, j:j + 1])
                if gamma is not None:
                    nc.vector.tensor_mul(y_g, y_g, gamma_sb[:])
                    nc.vector.tensor_add(y_g, y_g, beta_sb[:])

        nc.scalar.dma_start(out=out[r0:r0 + G * P, :].rearrange(
            "(g p) c -> p g c", p=P), in_=y_t[:])


def kernel(query, value, W_w, b_w, gamma, beta):
    global LAST_EXEC_NS, LAST_RESULTS

    query = np.ascontiguousarray(np.asarray(query, dtype=np.float32))
    value = np.ascontiguousarray(np.asarray(value, dtype=np.float32))
    W_w = np.ascontiguousarray(np.asarray(W_w, dtype=np.float32))
    b_w = np.asarray(b_w, dtype=np.float32)
    gamma = np.asarray(gamma, dtype=np.float32)
    beta = np.asarray(beta, dtype=np.float32)

    apply_affine = not (np.all(gamma == 1.0) and np.all(beta == 0.0))
    add_bias = bool(np.any(b_w != 0.0))

    key = (CONV_MODE, apply_affine, add_bias)
    if key not in _cache:
        _cache[key] = _build(*key)
    nc = _cache[key]

    in_maps = host_in_maps(query, value, W_w, CONV_MODE)
    if apply_affine:
        for m in in_maps:
            m["gamma"] = gamma.reshape(1, C)
            m["beta"] = beta.reshape(1, C)
    if add_bias:
        for m in in_maps:
            m["bw"] = b_w.reshape(1, K)

    res = run_bass_kernel_spmd(nc, in_maps, core_ids=list(range(N_CORES)),
                               trace=TRACE)
    LAST_EXEC_NS = res.exec_time_ns
    LAST_RESULTS = res
    out = np.empty((B, N, C), dtype=np.float32)
    for c in range(N_CORES):
        out[c * B_PER_CORE:(c + 1) * B_PER_CORE] = (
            res.results[c]["out"].reshape(B_PER_CORE, N, C))
    return out


def host_in_maps(query, value, W_w, mode):
    """Shard + lay out inputs for the 8 cores (host-side, layout only)."""
    qdt, vdt, _ = _dtypes(mode)
    qnp = ml_dtypes.bfloat16 if qdt == BF16 else np.float32
    vnp = ml_dtypes.bfloat16 if vdt == BF16 else np.float32

    wwt = np.ascontiguousarray(W_w.T.reshape(2, P, K)).astype(qnp)
    q_sh = query.reshape(N_CORES, ROWS, C)
    v_sh = value.reshape(N_CORES, ROWS, C)
    in_maps = []
    for c in range(N_CORES):
        in_maps.append({
            "qt": np.ascontiguousarray(q_sh[c].T).reshape(
                2, P, ROWS).astype(qnp),
            "v": np.ascontiguousarray(v_sh[c].astype(vnp)),
            "wwt": wwt,
        })
    return in_maps


# revision 22
# speedup vs baseline: 3.4466x; 1.3954x over previous
"""Trainium2 Bass kernel for DyDepthwiseConvAtten.

Computation (per (b, n) row r of C=256 channels):
  w[r, k]  = sum_c q[r, c] * W_w[k, c] + b_w[k]          (k = 0..2)
  x[r, c]  = sum_k w[r, k] * vpad[r, c + k]               (3-tap depthwise conv, 'same')
  out[r,c] = (x - mean_c(x)) * rsqrt(var_c(x) + eps) * gamma[c] + beta[c]

Strategy: pure data-parallel over batch across 8 cores. Rows (b*n flattened)
live on SBUF partitions; compute in tiles of 128 rows x 256 channels,
DMA-batched G row-tiles at a time (HWDGE descriptor-generation is ~625ns
per DMA instruction, so few big DMAs beat many small ones).

  - w via TensorE: host pre-transposes q so each tile's qT chunk [128c, 128r]
    is the (self-loading) stationary operand against W_w^T [128c, 3],
    accumulating the two c-chunks in PSUM.
  - conv via TensorE: x_psum += diag(w_k) @ v_shifted_k for k = 0..2, where
    diag(w_k) = identity * w_k is built with one per-partition tensor_scalar
    each.  In "f32r"/"bf16" modes the conv matmuls stream 1 column/cycle
    (4x the fp32 rate).
  - LayerNorm: bn_stats/bn_aggr on VectorE, sqrt on ScalarE, reciprocal on
    VectorE, normalize as one ScalarE activation y = Id(x * rs + (-mu * rs)).

Modes (BASS_DYCONV_MODE):
  f32   — everything fp32 (exact, conv matmuls at 4 cycles/row)
  f32r  — conv matmuls in float32r (fp32 bits, reduced-precision multiply)
  bf16  — value + conv in bf16 (halves v DMA traffic)
  bf16q — bf16 value/conv AND bf16 query/W_w (halves q DMA traffic too)
"""

import os
from contextlib import ExitStack

import numpy as np
import ml_dtypes

import concourse.bacc as bacc
import concourse.bass as bass
import concourse.tile as tile
from concourse import mybir
from concourse.bass_utils import run_bass_kernel_spmd
from concourse.masks import make_identity

B, N, C, K = 1024, 100, 256, 3
N_CORES = 8
B_PER_CORE = B // N_CORES        # 128
ROWS = B_PER_CORE * N            # 12800 rows per core
P = 128                          # partitions (rows per tile)
N_ROW_TILES = ROWS // P          # 100
G = int(os.environ.get("BASS_DYCONV_G", "10"))  # row-tiles per DMA super-tile
assert N_ROW_TILES % G == 0 and G % 2 == 0
LN_EPS = 1e-5
F32 = mybir.dt.float32
F32R = mybir.dt.float32r
BF16 = mybir.dt.bfloat16

CONV_MODE = os.environ.get("BASS_DYCONV_MODE", "f32")
TRACE = bool(int(os.environ.get("BASS_DYCONV_TRACE", "0")))

LAST_EXEC_NS = None
LAST_RESULTS = None

_cache = {}


def _dtypes(mode):
    """(q/wwt dtype, v dtype, dk dtype) per conv mode."""
    if mode == "f32":
        return F32, F32, F32
    if mode == "f32r":
        return F32, F32R, F32R
    if mode == "bf16":
        return F32, BF16, BF16
    if mode == "bf16q":
        return BF16, BF16, BF16
    raise ValueError(mode)


def _build(conv_mode: str, apply_affine: bool, add_bias: bool,
           loop_n: int = 1, ablate: str | None = None):
    nc = bacc.Bacc("TRN2", target_bir_lowering=False, debug=False)
    qdt, vdt, _ = _dtypes(conv_mode)

    # qT: per-core transposed query, chunked: [2, 128c, ROWS]
    qT = nc.dram_tensor("qt", [2, P, ROWS], qdt, kind="ExternalInput")
    v = nc.dram_tensor("v", [ROWS, C], vdt, kind="ExternalInput")
    # W_w^T chunks: [2, 128c, K]
    wwt = nc.dram_tensor("wwt", [2, P, K], qdt, kind="ExternalInput")
    out = nc.dram_tensor("out", [ROWS, C], F32, kind="ExternalOutput")
    gamma = beta = bwb = None
    if apply_affine:
        gamma = nc.dram_tensor("gamma", [1, C], F32, kind="ExternalInput")
        beta = nc.dram_tensor("beta", [1, C], F32, kind="ExternalInput")
    if add_bias:
        bwb = nc.dram_tensor("bw", [1, K], F32, kind="ExternalInput")

    def emit():
        _emit(ctx, tc, qT.ap(), v.ap(), wwt.ap(), out.ap(),
              gamma.ap() if gamma is not None else None,
              beta.ap() if beta is not None else None,
              bwb.ap() if bwb is not None else None,
              conv_mode, ablate)

    with tile.TileContext(nc) as tc, ExitStack() as ctx:
        if loop_n > 1:
            with tc.For_i(0, loop_n, 1):
                emit()
        else:
            emit()
    nc.compile()
    return nc


def _bcast_rows(ap: bass.AP, nrows: int) -> bass.AP:
    """DMA access pattern replicating a [1, F] DRAM tensor across partitions."""
    return bass.AP(tensor=ap.tensor, offset=ap.offset,
                   ap=[[0, nrows]] + list(ap.ap[1:]))


def _emit(ctx, tc, qT, v, wwt, out, gamma, beta, bwb, conv_mode,
          ablate=None):
    nc = tc.nc
    mult = mybir.AluOpType.mult
    AF = mybir.ActivationFunctionType
    qdt, vdt, kdt = _dtypes(conv_mode)

    singles = ctx.enter_context(tc.tile_pool(name="singles", bufs=1))
    pool = ctx.enter_context(tc.tile_pool(name="work", bufs=3))
    dkp = ctx.enter_context(tc.tile_pool(name="dkp", bufs=4))
    small = ctx.enter_context(tc.tile_pool(name="small", bufs=8))
    psum_x = ctx.enter_context(
        tc.tile_pool(name="psum_x", bufs=4, space=bass.MemorySpace.PSUM))
    psum_w = ctx.enter_context(
        tc.tile_pool(name="psum_w", bufs=3, space=bass.MemorySpace.PSUM))

    # one-time constants
    ident = singles.tile([P, P], F32)
    make_identity(nc, ident[:])
    wwt_sb = singles.tile([P, 2, K], qdt)
    nc.sync.dma_start(out=wwt_sb[:], in_=wwt.rearrange("a p k -> p a k"))
    eps_sb = singles.tile([P, 1], F32)
    nc.vector.memset(eps_sb[:], LN_EPS)
    if gamma is not None:
        gamma_sb = singles.tile([P, C], F32)
        nc.sync.dma_start(out=gamma_sb[:], in_=_bcast_rows(gamma, P))
        beta_sb = singles.tile([P, C], F32)
        nc.sync.dma_start(out=beta_sb[:], in_=_bcast_rows(beta, P))
    if bwb is not None:
        bw_sb = singles.tile([P, K], F32)
        nc.sync.dma_start(out=bw_sb[:], in_=_bcast_rows(bwb, P))
    if ablate == "nodk":
        dk_const = singles.tile([P, K, P], kdt)
        for k in range(K):
            nc.vector.tensor_scalar_mul(dk_const[:, k, :], ident[:],
                                        eps_sb[:])

    for st in range(N_ROW_TILES // G):
        r0 = st * G * P

        # ---- batched loads ----
        qt_t = pool.tile([P, 2, G * P], qdt, tag="qt")
        nc.sync.dma_start(out=qt_t[:],
                          in_=qT[:, :, r0:r0 + G * P].rearrange(
                              "a c r -> c a r"))
        v_t = pool.tile([P, G, C], vdt, tag="vt")
        nc.sync.dma_start(out=v_t[:],
                          in_=v[r0:r0 + G * P, :].rearrange(
                              "(g p) c -> p g c", p=P))
        y_t = pool.tile([P, G, C], F32, tag="y")

        # sub-tiles processed in pairs so the small LN ops (sqrt,
        # reciprocal) batch across two row-tiles; PSUM tiles stay
        # per-sub-tile (separate banks — same-bank PE-write/engine-read
        # is a hardware hazard)
        if ablate == "dma":
            for g in range(G):
                nc.scalar.copy(y_t[:, g, :], v_t[:, g, :])
            nc.scalar.dma_start(out=out[r0:r0 + G * P, :].rearrange(
                "(g p) c -> p g c", p=P), in_=y_t[:])
            continue

        for h in range(G // 2):
            xs = []
            mv = small.tile([P, 2, 2], F32, tag="mv")
            for j in range(2):
                g = 2 * h + j
                qt_g = qt_t[:, :, g * P:(g + 1) * P]
                v_g = v_t[:, g, :]

                if ablate == "now":
                    w_sb = eps_sb  # constant per-partition scalar
                else:
                    # dynamic weights w[r, k] on TensorE
                    w_ps = psum_w.tile([P, K], F32, tag="w_ps")
                    nc.tensor.matmul(w_ps[:], lhsT=qt_g[:, 0, :],
                                     rhs=wwt_sb[:, 0, :], start=True,
                                     stop=False)
                    nc.tensor.matmul(w_ps[:], lhsT=qt_g[:, 1, :],
                                     rhs=wwt_sb[:, 1, :], start=False,
                                     stop=True)
                    w_sb = small.tile([P, K], F32, tag="w_sb")
                    if bwb is not None:
                        nc.vector.tensor_add(w_sb[:], w_ps[:], bw_sb[:])
                    else:
                        nc.scalar.copy(w_sb[:], w_ps[:])

                # diagonal stationaries diag(w_k) = identity * w_k
                if ablate == "nodk":
                    dk = dk_const
                else:
                    dk = dkp.tile([P, K, P], kdt, tag="dk")
                    for k in range(K):
                        nc.vector.tensor_scalar_mul(
                            dk[:, k, :], ident[:],
                            w_sb[:, k if ablate != "now" else 0:
                                 (k if ablate != "now" else 0) + 1])
                # 'same' padding without a padded buffer: the aligned k=1
                # tap covers all C columns (start=True clears PSUM); k=0 /
                # k=2 accumulate over their valid C-1 column subranges.
                x_ps = psum_x.tile([P, C], F32, tag="x_ps")
                nc.tensor.matmul(x_ps[:], lhsT=dk[:, 1, :], rhs=v_g[:],
                                 start=True, stop=False)
                nc.tensor.matmul(x_ps[:, 1:C], lhsT=dk[:, 0, :],
                                 rhs=v_g[:, 0:C - 1], start=False,
                                 stop=False)
                nc.tensor.matmul(x_ps[:, 0:C - 1], lhsT=dk[:, 2, :],
                                 rhs=v_g[:, 1:C], start=False, stop=True)
                xs.append(x_ps)

                if ablate == "noln":
                    continue
                # LayerNorm stats
                stats = small.tile([P, 6], F32, tag="stats")
                nc.vector.bn_stats(out=stats[:], in_=x_ps[:])
                nc.vector.bn_aggr(out=mv[:, j, :], in_=stats[:])

            if ablate == "noln":
                for j in range(2):
                    nc.scalar.copy(y_t[:, 2 * h + j, :], xs[j][:])
                continue
            # batched small LN ops across the pair
            rs = small.tile([P, 2], F32, tag="rs")
            nc.scalar.activation(rs[:], mv[:, :, 1], AF.Sqrt, bias=eps_sb[:])
            nc.vector.reciprocal(rs[:], rs[:])
            nbias = small.tile([P, 2], F32, tag="nbias")
            for j in range(2):
                nc.vector.tensor_scalar(out=nbias[:, j:j + 1],
                                        in0=mv[:, j, 0:1],
                                        scalar1=rs[:, j:j + 1],
                                        scalar2=-1.0, op0=mult, op1=mult)
            for j in range(2):
                g = 2 * h + j
                y_g = y_t[:, g, :]
                nc.scalar.activation(y_g, xs[j][:], AF.Identity,
                                     bias=nbias[:, j:j + 1],
                                     scale=rs[:, j:j + 1])
                if gamma is not None:
                    nc.vector.tensor_mul(y_g, y_g, gamma_sb[:])
                    nc.vector.tensor_add(y_g, y_g, beta_sb[:])

        nc.scalar.dma_start(out=out[r0:r0 + G * P, :].rearrange(
            "(g p) c -> p g c", p=P), in_=y_t[:])


def kernel(query, value, W_w, b_w, gamma, beta):
    global LAST_EXEC_NS, LAST_RESULTS

    query = np.ascontiguousarray(np.asarray(query, dtype=np.float32))
    value = np.ascontiguousarray(np.asarray(value, dtype=np.float32))
    W_w = np.ascontiguousarray(np.asarray(W_w, dtype=np.float32))
    b_w = np.asarray(b_w, dtype=np.float32)
    gamma = np.asarray(gamma, dtype=np.float32)
    beta = np.asarray(beta, dtype=np.float32)

    apply_affine = not (np.all(gamma == 1.0) and np.all(beta == 0.0))
    add_bias = bool(np.any(b_w != 0.0))

    key = (CONV_MODE, apply_affine, add_bias)
    if key not in _cache:
        _cache[key] = _build(*key)
    nc = _cache[key]

    in_maps = host_in_maps(query, value, W_w, CONV_MODE)
    if apply_affine:
        for m in in_maps:
            m["gamma"] = gamma.reshape(1, C)
            m["beta"] = beta.reshape(1, C)
    if add_bias:
        for m in in_maps:
            m["bw"] = b_w.reshape(1, K)

    res = run_bass_kernel_spmd(nc, in_maps, core_ids=list(range(N_CORES)),
                               trace=TRACE)
    LAST_EXEC_NS = res.exec_time_ns
    LAST_RESULTS = res
    out = np.empty((B, N, C), dtype=np.float32)
    for c in range(N_CORES):
        out[c * B_PER_CORE:(c + 1) * B_PER_CORE] = (
            res.results[c]["out"].reshape(B_PER_CORE, N, C))
    return out


def host_in_maps(query, value, W_w, mode):
    """Shard + lay out inputs for the 8 cores (host-side, layout only)."""
    qdt, vdt, _ = _dtypes(mode)
    qnp = ml_dtypes.bfloat16 if qdt == BF16 else np.float32
    vnp = ml_dtypes.bfloat16 if vdt == BF16 else np.float32

    wwt = np.ascontiguousarray(W_w.T.reshape(2, P, K)).astype(qnp)
    q_sh = query.reshape(N_CORES, ROWS, C)
    v_sh = value.reshape(N_CORES, ROWS, C)
    in_maps = []
    for c in range(N_CORES):
        in_maps.append({
            "qt": np.ascontiguousarray(q_sh[c].T).reshape(
                2, P, ROWS).astype(qnp),
            "v": np.ascontiguousarray(v_sh[c].astype(vnp)),
            "wwt": wwt,
        })
    return in_maps
